# revision 5
# baseline (speedup 1.0000x reference)
"""Trainium2 kernel for nn_CenterDisc (segment_reduce).

Computes: per-class (4 classes) mean of x rows (N=4096 rows of 64x512),
then mean pairwise Frobenius distance between the 4 class centers.

Strategy (data-parallel over N, 8 cores):
  - host: shard x rows 512/core and pre-arrange each shard into
    block-contiguous layout: block c = [128 part, 4 k-chunks x Wc cols]
    so each x DMA is one fully contiguous 4 MB read (measured ~393 GB/s
    per-NC vs ~375 GB/s for the 1 MB strided pattern).
  - device: per-class partial sums via TensorE matmul
        sums[cls, d] = sum_k onehot[k, cls] * x[k, d]
    onehot [128, 4] is the stationary operand, accumulation over the 4
    row-chunks lands in one wide PSUM tile [4, Wc] (bank-aligned 512
    slices), one DVE copy per block evicts PSUM -> SBUF, out-DMAs ride
    the scalar ring. Tail blocks taper 2M/1M/0.5M/0.5M so almost no
    compute remains after the last DMA byte.
  - host: add the 8 partial (4, 32768) sums, counts = bincount(labels),
    centers + pairwise norms (tiny) on host.
"""

import numpy as np

import concourse.bass as bass
import concourse.tile as tile
from concourse import bacc, mybir
from concourse.bass import ts
from concourse.bass_utils import run_bass_kernel_spmd

# Problem shape (hardcoded per contract)
N, C, PDIM = 4096, 64, 512
D = C * PDIM           # 32768 features per row
NCLS = 4               # num classes
CORES = 8
R = N // CORES         # 512 rows per core
KP = 128               # rows per matmul chunk (partition dim)
KC = R // KP           # 4 k-chunks per core
MM = 512               # matmul moving free dim (fp32 max / PSUM bank)

# Output-column widths per block; DMA bytes per block = W * KC * KP * 4
# = W * 2048 bytes. 15 x 4MB, then tapered tail.
WIDTHS = [2048] * 15 + [1024, 512, 256, 256]
assert sum(WIDTHS) == D

_NC_CACHE = None


def _build_bass():
    nc = bacc.Bacc()
    # float32r: same 4-byte layout as fp32 (host arrays stay np.float32),
    # but the PE streams it ~2x faster than fp32's 4 cycles/row.
    mm_dt = mybir.dt.float32r
    x_in = nc.dram_tensor("x", [R * D], mm_dt, kind="ExternalInput")
    oh_in = nc.dram_tensor("onehot", [R, NCLS], mm_dt,
                           kind="ExternalInput")
    out = nc.dram_tensor("sums", [NCLS, D], mybir.dt.float32,
                         kind="ExternalOutput")

    oh_r = oh_in[:, :].rearrange("(k p) c -> k p c", p=KP)    # (KC, 128, NCLS)

    with tile.TileContext(nc) as tc:
        with (
            tc.tile_pool(name="ohp", bufs=1) as ohp,
            tc.tile_pool(name="xp", bufs=4) as xp,
            tc.tile_pool(name="outp", bufs=2) as outp,
            tc.tile_pool(name="pp", bufs=2, space="PSUM") as pp,
        ):
            ohts = []
            for k in range(KC):
                t = ohp.tile([KP, NCLS], mm_dt, tag=f"oh{k}")
                nc.scalar.dma_start(out=t[:], in_=oh_r[k])
                ohts.append(t)

            col = 0
            off = 0
            for bi, w in enumerate(WIDTHS):
                xt = xp.tile([KP, KC * w], mm_dt, tag="x")
                nc.sync.dma_start(
                    out=xt[:],
                    in_=x_in[off:off + KP * KC * w].rearrange(
                        "(p c) -> p c", p=KP))
                off += KP * KC * w
                ps = pp.tile([NCLS, w], mybir.dt.float32, tag="ps",
                             name=f"ps{bi}")
                js = max(1, w // MM)
                sw = w // js
                for k in range(KC):
                    for j in range(js):
                        nc.tensor.matmul(
                            ps[:, ts(j, sw)],
                            ohts[k][:],
                            xt[:, k * w + j * sw:k * w + (j + 1) * sw],
                            start=(k == 0),
                            stop=(k == KC - 1),
                        )
                ot = outp.tile([NCLS, w], mybir.dt.float32, tag="ot")
                nc.vector.tensor_copy(out=ot[:], in_=ps[:])
                nc.scalar.dma_start(out=out[:, col:col + w], in_=ot[:])
                col += w
    nc.compile()
    return nc


def _get_nc():
    global _NC_CACHE
    if _NC_CACHE is None:
        _NC_CACHE = _build_bass()
    return _NC_CACHE


def _prearrange(xs):
    """xs: (R, D) core shard -> flat (R*D,) block-major layout.

    Block bi holds output cols [col, col+w): its KP*KC*w floats are laid
    out as [p, k, j] so partition p of the [KP, KC*w] tile is the
    contiguous run  concat_k x[k*KP + p, col:col+w].
    """
    out = np.empty(R * D, dtype=np.float32)
    xk = xs.reshape(KC, KP, D)
    col = 0
    off = 0
    for w in WIDTHS:
        n = KP * KC * w
        out[off:off + n] = (
            xk[:, :, col:col + w].transpose(1, 0, 2).reshape(-1))
        col += w
        off += n
    return out


def _run(x, labels, trace=False, **spmd_kwargs):
    x = np.asarray(x, dtype=np.float32).reshape(N, D)
    labels = np.asarray(labels).astype(np.int64)
    onehot = (labels[:, None] == np.arange(NCLS)[None, :]).astype(np.float32)

    in_maps = [
        {"x": _prearrange(x[c * R:(c + 1) * R]),
         "onehot": np.ascontiguousarray(onehot[c * R:(c + 1) * R])}
        for c in range(CORES)
    ]
    nc = _get_nc()
    last_err = None
    for attempt in range(3):
        try:
            br = run_bass_kernel_spmd(nc, in_maps, core_ids=list(range(CORES)),
                                      trace=trace, **spmd_kwargs)
            break
        except Exception as e:  # transient device wedge (NRT_*) — retry
            last_err = e
            import time as _time
            _time.sleep(3.0)
    else:
        raise last_err

    sums = np.zeros((NCLS, D), dtype=np.float64)
    for r in br.results:
        sums += r["sums"].astype(np.float64)
    counts = np.bincount(labels, minlength=NCLS).astype(np.float64)
    safe = np.maximum(counts, 1.0)
    centers = sums / safe[:, None]                         # (NCLS, D)
    diffs = centers[:, None, :] - centers[None, :, :]      # (NCLS, NCLS, D)
    norms = np.sqrt(np.sum(diffs * diffs, axis=-1))        # (NCLS, NCLS)
    iu, ju = np.triu_indices(NCLS, k=1)
    distance = np.sum(norms[iu, ju]) / len(iu)
    return np.asarray(distance, dtype=np.float32), br


def kernel(x, labels):
    result, _ = _run(x, labels, trace=False)
    return result


# revision 8
# speedup vs baseline: 1.0279x; 1.0279x over previous
"""Trainium2 kernel for nn_CenterDisc (segment_reduce).

Computes: per-class (4 classes) mean of x rows (N=4096 rows of 64x512),
then mean pairwise Frobenius distance between the 4 class centers.

Strategy (data-parallel over N, 8 cores):
  - host: shard x rows 512/core and pre-arrange each shard into
    block-contiguous layout: block c = [128 part, 4 k-chunks x Wc cols]
    so each x DMA is one fully contiguous 4 MB read (measured ~393 GB/s
    per-NC vs ~375 GB/s for the 1 MB strided pattern).
  - device: per-class partial sums via TensorE matmul
        sums[cls, d] = sum_k onehot[k, cls] * x[k, d]
    onehot [128, 4] is the stationary operand, accumulation over the 4
    row-chunks lands in one wide PSUM tile [4, Wc] (bank-aligned 512
    slices), one DVE copy per block evicts PSUM -> SBUF, out-DMAs ride
    the scalar ring. Tail blocks taper 2M/1M/0.5M/0.5M so almost no
    compute remains after the last DMA byte.
  - host: add the 8 partial (4, 32768) sums, counts = bincount(labels),
    centers + pairwise norms (tiny) on host.
"""

import numpy as np

import concourse.bass as bass
import concourse.tile as tile
from concourse import bacc, mybir
from concourse.bass import ts
from concourse.bass_utils import run_bass_kernel_spmd

# Problem shape (hardcoded per contract)
N, C, PDIM = 4096, 64, 512
D = C * PDIM           # 32768 features per row
NCLS = 4               # num classes
CORES = 8
R = N // CORES         # 512 rows per core
KP = 128               # rows per matmul chunk (partition dim)
KC = R // KP           # 4 k-chunks per core
MM = 512               # matmul moving free dim (fp32 max / PSUM bank)

# Output-column widths per block; DMA bytes per block = W * KC * KP * 4
# = W * 2048 bytes. 15 x 4MB, then tapered tail.
WIDTHS = [2048] * 15 + [1024, 512, 256, 256]
assert sum(WIDTHS) == D

_NC_CACHE = None


def _build_bass():
    nc = bacc.Bacc()
    # float32r: same 4-byte layout as fp32 (host arrays stay np.float32),
    # but the PE streams it ~2x faster than fp32's 4 cycles/row.
    mm_dt = mybir.dt.float32r
    x_in = nc.dram_tensor("x", [R * D], mm_dt, kind="ExternalInput")
    # blocked one-hot: row p, col k*NCLS + c = (labels[k*KP + p] == c)
    oh_in = nc.dram_tensor("onehot", [KP, KC * NCLS], mm_dt,
                           kind="ExternalInput")
    out = nc.dram_tensor("sums", [NCLS, D], mybir.dt.float32,
                         kind="ExternalOutput")

    with tile.TileContext(nc) as tc:
        with (
            tc.tile_pool(name="ohp", bufs=1) as ohp,
            tc.tile_pool(name="xp", bufs=4) as xp,
            tc.tile_pool(name="outp", bufs=2) as outp,
            tc.tile_pool(name="pp", bufs=2, space="PSUM") as pp,
        ):
            # One DMA for all 4 one-hots, first on the sync ring so it
            # completes before x-block 0 (separate tiny DMAs got their
            # completion sems entangled with the x stream: 40us stall).
            oht = ohp.tile([KP, KC * NCLS], mm_dt, tag="oh")
            nc.sync.dma_start(out=oht[:], in_=oh_in[:, :])
            ohts = [oht[:, k * NCLS:(k + 1) * NCLS] for k in range(KC)]

            col = 0
            off = 0
            for bi, w in enumerate(WIDTHS):
                xt = xp.tile([KP, KC * w], mm_dt, tag="x")
                nc.sync.dma_start(
                    out=xt[:],
                    in_=x_in[off:off + KP * KC * w].rearrange(
                        "(p c) -> p c", p=KP))
                off += KP * KC * w
                ps = pp.tile([NCLS, w], mybir.dt.float32, tag="ps",
                             name=f"ps{bi}")
                js = max(1, w // MM)
                sw = w // js
                for k in range(KC):
                    for j in range(js):
                        nc.tensor.matmul(
                            ps[:, ts(j, sw)],
                            ohts[k],
                            xt[:, k * w + j * sw:k * w + (j + 1) * sw],
                            start=(k == 0),
                            stop=(k == KC - 1),
                        )
                ot = outp.tile([NCLS, w], mybir.dt.float32, tag="ot")
                nc.vector.tensor_copy(out=ot[:], in_=ps[:])
                nc.scalar.dma_start(out=out[:, col:col + w], in_=ot[:])
                col += w
    nc.compile()
    return nc


def _get_nc():
    global _NC_CACHE
    if _NC_CACHE is None:
        _NC_CACHE = _build_bass()
    return _NC_CACHE


def _prearrange(xs):
    """xs: (R, D) core shard -> flat (R*D,) block-major layout.

    Block bi holds output cols [col, col+w): its KP*KC*w floats are laid
    out as [p, k, j] so partition p of the [KP, KC*w] tile is the
    contiguous run  concat_k x[k*KP + p, col:col+w].
    """
    out = np.empty(R * D, dtype=np.float32)
    xk = xs.reshape(KC, KP, D)
    col = 0
    off = 0
    for w in WIDTHS:
        n = KP * KC * w
        out[off:off + n] = (
            xk[:, :, col:col + w].transpose(1, 0, 2).reshape(-1))
        col += w
        off += n
    return out


def _run(x, labels, trace=False, **spmd_kwargs):
    x = np.asarray(x, dtype=np.float32).reshape(N, D)
    labels = np.asarray(labels).astype(np.int64)
    # blocked one-hot per core: (KP, KC*NCLS), col k*NCLS+c for chunk k
    lab_k = labels.reshape(CORES, KC, KP)
    oh = (lab_k[..., None] == np.arange(NCLS)).astype(np.float32)
    oh_blocked = oh.transpose(0, 2, 1, 3).reshape(CORES, KP, KC * NCLS)

    in_maps = [
        {"x": _prearrange(x[c * R:(c + 1) * R]),
         "onehot": np.ascontiguousarray(oh_blocked[c])}
        for c in range(CORES)
    ]
    nc = _get_nc()
    last_err = None
    for attempt in range(3):
        try:
            br = run_bass_kernel_spmd(nc, in_maps, core_ids=list(range(CORES)),
                                      trace=trace, **spmd_kwargs)
            break
        except Exception as e:  # transient device wedge (NRT_*) — retry
            last_err = e
            import time as _time
            _time.sleep(3.0)
    else:
        raise last_err

    sums = np.zeros((NCLS, D), dtype=np.float64)
    for r in br.results:
        sums += r["sums"].astype(np.float64)
    counts = np.bincount(labels, minlength=NCLS).astype(np.float64)
    safe = np.maximum(counts, 1.0)
    centers = sums / safe[:, None]                         # (NCLS, D)
    diffs = centers[:, None, :] - centers[None, :, :]      # (NCLS, NCLS, D)
    norms = np.sqrt(np.sum(diffs * diffs, axis=-1))        # (NCLS, NCLS)
    iu, ju = np.triu_indices(NCLS, k=1)
    distance = np.sum(norms[iu, ju]) / len(iu)
    return np.asarray(distance, dtype=np.float32), br


def kernel(x, labels):
    result, _ = _run(x, labels, trace=False)
    return result
